# revision 13
# baseline (speedup 1.0000x reference)
"""Trainium2 Bass kernel: single-channel Conv2d.

  x: [32, 224, 224] f32, kernels: [64, 7, 7] f32
  out[b, k, i, j] = sum_{di,dj} x[b, i+di, j+dj] * kernels[k, di, dj]
  -> [32, 64, 218, 218]

Sharding: data-parallel over batch, 4 images (= 2 image-pairs q) per core.

Layout (v4): matmul OUTPUT partitions are (img, k) = 2*64 = 128 and the
stream is a single output row j = 0..223, so every store DMA is fully
contiguous per (img, k) in DRAM (nrows*218-float runs, 3.5-24 KB
descriptors; the v1 baseline's 872 B descriptors made DMA descriptor-
overhead-bound at ~120 GB/s).

  - Host ships x re-laid-out as xs[q, seg, 2*p+img, j] = x[2q+img,
    rlo[seg]+p, j]  (5 segs of 64 rows per image-pair, images interleaved
    in partitions so a 16-row gather read spans 8 DMA ports instead of 4),
    zero-padded to 240 cols; and banded stationary weights
    w3h[128, (r*2+dd)*128 + img*64 + k]:
      row c = img'*64 + dr*4 + g -> delta(img'==img) * w[k, dr-r, g+4*dd]
    (zero outside 0<=dr-r<7, g+4*dd<7).  Both ship as raw fp32 bits into
    float32r tensors (f32r = fp32 bits; PE rounds the mantissa, streams at
    full PE rate).
  - Per block of 10 output rows (r0 = 10b, last block r0=208 partial): one
    [64, 232] gather DMA per img with in-AP ((480,16),(1,4),(1,232)) -- the
    (1,4) dim materializes the 4 column shifts; then per row r: 2
    accumulating matmuls (dd = 0,1; taps dj = g+4*dd) with stationary
    w3[r*2+dd] -> PSUM [128 = 2 img x 64 ch, 224].
  - VectorE (cols 0:138) + ScalarE (cols 138:218) evacuate PSUM into
    chunks of ramped size [4, 8, 16, 28, ...] rows (ramp -> first store
    DMA issues early); one store DMA per chunk:
      SBUF ((6104,128),(1,nrows*218)) -> DRAM ((OIMG,2),(HO*WO,64),(1,nrows*218)).
  - Queues: loads + gathers on SP (qSyncDynamicHW); stores on ACT
    (qScalarDynamicHW); no SWDGE (avoids the DVE/GpSimd shared-port trap).
"""
import sys

sys.path.insert(0, "/opt/trn_rl_repo")

import numpy as np

B, H, W = 32, 224, 224
KCH, KS = 64, 7
HO = WO = H - KS + 1  # 218
NCORES = 8
BLOC = B // NCORES    # 4 images per core
NQ = BLOC // 2        # 2 image-pairs per core
RB = 16               # x-rows per gather block
NRB = 10              # output rows per full block
NBLK = 22             # blocks per image-pair (21 full + 1 partial)
NSEG = 5
SEG_RLO = (0, 48, 96, 144, 160)   # seg -> first x row (64 rows each)
SEG_OF_BLOCK = (0, 0, 0, 0, 0, 1, 1, 1, 1, 1, 2, 2, 2, 2, 2,
                3, 3, 3, 3, 3, 4, 4)
XSW = 240             # xs cols per row (224 + 16 zero pad)
PTW = 232
NST = 224             # matmul stream length
DVE_COLS = 138        # PSUM evacuation split: VectorE cols, rest ScalarE
CROWS = 28            # max rows per output SBUF chunk
CHUNK_SIZES = (4, 8, 16, 28, 28, 28, 28, 28, 28, 22)   # sums to 218
OIMG = KCH * HO * WO
HOWO = HO * WO

_NC_CACHE = {}


def make_w3(kernels: np.ndarray) -> np.ndarray:
    """Banded stationary matrices [128, 20*128].
    w3h[img*64 + dr*4 + g, (r*2+dd)*128 + img*64 + k] = kernels[k, dr-r, g+4dd]
    for 0 <= dr-r < 7 and g+4dd < 7, else 0."""
    w3 = np.zeros((10, 2, 128, 128), dtype=np.float32)
    for r in range(10):
        for dd in range(2):
            for dr in range(RB):
                di = dr - r
                if not (0 <= di < KS):
                    continue
                for g in range(4):
                    dj = g + 4 * dd
                    if dj >= KS:
                        continue
                    for img in range(2):
                        w3[r, dd, img * 64 + dr * 4 + g,
                           img * 64: img * 64 + KCH] = kernels[:, di, dj]
    return np.ascontiguousarray(
        w3.transpose(2, 0, 1, 3).reshape(128, 20 * 128))


def make_xs(xc: np.ndarray) -> np.ndarray:
    """Interleaved input segs [NQ, NSEG, 128, XSW] for one core's xc [4,H,W].
    xs[q, s, 2*p + img, j] = xc[2q+img, SEG_RLO[s]+p, j], zero-padded."""
    xs = np.zeros((NQ, NSEG, 128, XSW), dtype=np.float32)
    for s in range(NSEG):
        rlo = SEG_RLO[s]
        blk = xc[:, rlo: rlo + 64, :]            # [4, 64, 224]
        for img in range(2):
            xs[:, s, img::2, :W] = blk[img::2]   # [NQ, 64, 224]
    return xs


def _build_nc():
    import concourse.bacc as bacc
    import concourse.mybir as mybir
    import concourse.tile as tile
    from concourse.bass_types import AP

    F32 = mybir.dt.float32
    F32R = mybir.dt.float32r

    nc = bacc.Bacc("TRN2", target_bir_lowering=False, debug=False,
                   num_devices=NCORES)
    xs_d = nc.dram_tensor("xs", [NQ, NSEG, 128, XSW], F32R,
                          kind="ExternalInput").ap()
    w3_d = nc.dram_tensor("w3", [128, 20 * 128], F32R,
                          kind="ExternalInput").ap()
    out_d = nc.dram_tensor("out", [BLOC, KCH, HO, WO], F32,
                           kind="ExternalOutput").ap()

    with tile.TileContext(nc) as tc:
        with (
            tc.tile_pool(name="wpool", bufs=1) as wpool,
            tc.tile_pool(name="xspool", bufs=7) as xspool,
            tc.tile_pool(name="ptpool", bufs=12) as ptpool,
            tc.tile_pool(name="opool", bufs=3) as opool,
            tc.tile_pool(name="psum", bufs=8, space="PSUM") as psum,
        ):
            # HAM warm-up: ~4.3us of junk matmuls on a never-written tile
            # (no deps -> issue at preamble end) so the PE clock gate is
            # at 8/8 when the first real matmul arrives.
            dummy = wpool.tile([128, 512], F32)
            nc.gpsimd.memset(dummy[:], 0.0)
            psw = psum.tile([128, 512], F32, tag="ps")
            for _ in range(10):
                nc.tensor.matmul(out=psw[:],
                                 lhsT=dummy[:, 0:128].bitcast(F32R),
                                 rhs=dummy[:].bitcast(F32R),
                                 start=True, stop=True)
            wfr_a = wpool.tile([128, 512], F32R)    # r = 0, 1
            wfr_b = wpool.tile([128, 2048], F32R)   # r = 2..9
            # seg s first needed at block 5s; prefetch ~5 blocks early
            load_before = {0: [0], 1: [1], 5: [2], 10: [3], 15: [4]}
            for q in range(NQ):
                xseg = [None] * NSEG
                gi = 0
                ci = 0          # chunk index within q
                chunk = None
                crow0 = 0
                nrows_c = 0
                for b in range(NBLK):
                    for s in load_before.get(b, ()):
                        xt_new = xspool.tile([128, XSW], F32R, tag="xs")
                        # seg loads go on the ACT ring so they never delay
                        # gathers in the SP FIFO
                        eng = nc.sync if (q == 0 and b == 0) else nc.scalar
                        eng.dma_start(out=xt_new[:], in_=xs_d[q, s])
                        xseg[s] = xt_new
                        if q == 0 and b == 0:
                            # W r=0,1 tile first: unblocks block 0's
                            # matmuls; bulk follows behind gather b0.
                            nc.sync.dma_start(out=wfr_a[:],
                                              in_=w3_d[:, 0:512])
                    r0 = NRB * b if b < NBLK - 1 else 208
                    s = SEG_OF_BLOCK[b]
                    xt = xseg[s]
                    pb = 2 * (r0 - SEG_RLO[s])   # partition of (row r0, img0)
                    pt = ptpool.tile([128, PTW], F32R, tag="pt")
                    for img in range(2):
                        nc.sync.dma_start(
                            out=pt[img * 64: (img + 1) * 64, :],
                            in_=AP(
                                tensor=xt[:].tensor,
                                offset=xt[:].offset + (pb + img) * XSW,
                                ap=((2 * XSW, RB), (1, 4), (1, PTW)),
                            ),
                        )
                    if q == 0 and b == 0:
                        nc.sync.dma_start(out=wfr_b[:],
                                          in_=w3_d[:, 512:2560])
                    rlist = range(NRB) if b < NBLK - 1 else range(2, NRB)
                    for r in rlist[::2]:
                        # two output rows share one PSUM bank
                        ps = psum.tile([128, 512], F32, tag="ps")
                        for p2 in range(2):
                            for dd in range(2):
                                t = ((r + p2) * 2 + dd) * 128
                                wt = (wfr_a[:, t: t + 128] if t < 4 * 128
                                      else wfr_b[:, t - 512: t - 512 + 128])
                                nc.tensor.matmul(
                                    out=ps[:, p2 * NST: (p2 + 1) * NST],
                                    lhsT=wt,
                                    rhs=pt[:, 4 * dd: 4 * dd + NST],
                                    start=(dd == 0), stop=(dd == 1),
                                )
                        slot = gi - crow0
                        if chunk is None:
                            nrows_c = CHUNK_SIZES[ci]
                            chunk = opool.tile([128, CROWS * WO], F32,
                                               tag="osb")
                        # 2-row evacuation, 3-dim APs
                        nc.vector.tensor_copy(
                            out=AP(tensor=chunk[:].tensor,
                                   offset=chunk[:].offset + slot * WO,
                                   ap=((CROWS * WO, 128), (WO, 2),
                                       (1, DVE_COLS))),
                            in_=AP(tensor=ps[:].tensor,
                                   offset=ps[:].offset,
                                   ap=((512, 128), (NST, 2),
                                       (1, DVE_COLS))))
                        nc.scalar.copy(
                            out=AP(tensor=chunk[:].tensor,
                                   offset=chunk[:].offset + slot * WO
                                   + DVE_COLS,
                                   ap=((CROWS * WO, 128), (WO, 2),
                                       (1, WO - DVE_COLS))),
                            in_=AP(tensor=ps[:].tensor,
                                   offset=ps[:].offset + DVE_COLS,
                                   ap=((512, 128), (NST, 2),
                                       (1, WO - DVE_COLS))))
                        if slot == nrows_c - 2:
                            st_in = AP(
                                tensor=chunk[:].tensor,
                                offset=chunk[:].offset,
                                ap=((CROWS * WO, 128), (1, nrows_c * WO)),
                            )
                            st_out = AP(
                                tensor=out_d.tensor,
                                offset=2 * q * OIMG + crow0 * WO,
                                ap=((OIMG, 2), (HOWO, KCH),
                                    (1, nrows_c * WO)),
                            )
                            nc.scalar.dma_start(out=st_out, in_=st_in)
                            chunk = None
                            ci += 1
                            crow0 = gi + 2
                        gi += 2
    nc.compile()
    return nc


def _get_nc():
    if "nc" not in _NC_CACHE:
        _NC_CACHE["nc"] = _build_nc()
    return _NC_CACHE["nc"]


def _run(x: np.ndarray, kernels: np.ndarray, **kw):
    from concourse.bass_utils import run_bass_kernel_spmd

    x = np.ascontiguousarray(np.asarray(x, dtype=np.float32))
    kernels = np.ascontiguousarray(np.asarray(kernels, dtype=np.float32))
    w3h = make_w3(kernels)
    nc = _get_nc()
    in_maps = [
        {"xs": make_xs(x[c * BLOC: (c + 1) * BLOC]), "w3": w3h}
        for c in range(NCORES)
    ]
    return run_bass_kernel_spmd(nc, in_maps, core_ids=list(range(NCORES)),
                                **kw)


def kernel(x: np.ndarray, kernels: np.ndarray) -> np.ndarray:
    res = _run(x, kernels)
    return np.concatenate([res.results[c]["out"] for c in range(NCORES)],
                          axis=0)
